# revision 27
# baseline (speedup 1.0000x reference)
"""Multi-head self-attention (RoPE, causal) Trainium2 Bass kernel, 8 NeuronCores.

Sharding: data-parallel over batch (B=2) x tensor-parallel over heads
(16 heads -> 4 groups of 4). Core c handles batch b=c//4, heads 4*(c%4)..4*(c%4)+3.
Each core computes its 4 heads' attention plus a partial output projection;
the host sums the 4 partial outputs per batch element.

Single fused software-pipeline over 512-wide q-chunks: chunk qc's attention
interleaves the *emission* of chunk qc+1's Q/K/V projection matmuls (and the
out-projection of qc-2) as tensor-queue filler, so the tensor engine never
stalls on the scalar-engine exp and output DMA spreads across the kernel.

Attention layout: scores in [k, q] (T = K^T.T @ Q^T, 64-deep contraction,
2 heads PE-packed per PSUM tile; P^T = exp(T/8), one wide activation per
head-pair), but PV in [q, d]: P^T 128x128 blocks are the matmul *stationary*
(M=128 q) against moving [V_h | 1] (N=65), halving PV row cost vs the [d, q]
form and making the softmax denominator a per-partition scalar (free column
64) - normalization is a reciprocal on [128,1] plus a per-partition-scaled
copy, no broadcasts. Non-diagonal PV lags scores by 2 iterations (exp slack);
the masked diagonal-block PV lags by 4 (mask slack). PSUM accumulation rule:
start=True logically zeroes the whole (partition-range x bank) region, so
only the first PV matmul into a po bank carries start=True. A per head-pair
is PE-transposed ([q, 2x64d] -> [128ch, q]) into `at` for the out-projection.
"""
import sys, math

sys.path.insert(0, "/opt/trn_rl_repo")

import numpy as np
import ml_dtypes

import concourse.bacc as bacc
import concourse.bass as bass
import concourse.mybir as mybir
import concourse.tile as tile
from concourse.bass_utils import run_bass_kernel_spmd

BF16 = mybir.dt.bfloat16
F32 = mybir.dt.float32
FP8 = mybir.dt.float8e4
NPBF16 = ml_dtypes.bfloat16
NPFP8 = ml_dtypes.float8_e4m3fn

D_MODEL = 1024
D_HEAD = 64
HALF = D_HEAD // 2
ROPE_THETA = 10000.0
N_CORES = 8
C = 256  # channels per core (4 heads x 64)
SWAP32 = [i ^ 1 for i in range(32)]

FP8_QK = False  # fp8e4 DoubleRow Q/K projection: fails the 2e-2 gate (5.6e-2)


def _body(nc, tc, L, pp, rtp, ptp, a2p, rip, osp):
    n_lt = L // 128
    qw = 512
    n_qch = L // qw
    scale = 1.0 / math.sqrt(D_HEAD)

    xt_d = nc.dram_tensor("xt", [D_MODEL, L], BF16, kind="ExternalInput").ap()
    wq_d = nc.dram_tensor("wqt", [D_MODEL, C],
                          FP8 if FP8_QK else BF16, kind="ExternalInput").ap()
    wk_d = nc.dram_tensor("wkt", [D_MODEL, C],
                          FP8 if FP8_QK else BF16, kind="ExternalInput").ap()
    wv_d = nc.dram_tensor("wvt", [D_MODEL, C], BF16, kind="ExternalInput").ap()
    wo_d = nc.dram_tensor("wot", [C, D_MODEL], BF16, kind="ExternalInput").ap()
    cos_d = nc.dram_tensor("cosb", [128, L], BF16, kind="ExternalInput").ap()
    sin_d = nc.dram_tensor("ssin", [128, L], BF16, kind="ExternalInput").ap()
    mk_d = nc.dram_tensor("masks", [128, 128], BF16, kind="ExternalInput").ap()
    id_d = nc.dram_tensor("ident", [128, 128], BF16, kind="ExternalInput").ap()
    out_d = nc.dram_tensor("out", [L, D_MODEL], BF16, kind="ExternalOutput").ap()
    if FP8_QK:
        x8_d = nc.dram_tensor("xt8", [D_MODEL, L], FP8, kind="ExternalInput").ap()

    # ---- persistent SBUF tensors
    wdt = FP8 if FP8_QK else BF16
    wq = pp.tile([128, 8, C], wdt)
    wk = pp.tile([128, 8, C], wdt)
    wv = pp.tile([128, 8, C], BF16)
    wo = pp.tile([128, 2, D_MODEL], BF16)
    cs = pp.tile([128, L], BF16)
    sn = pp.tile([128, L], BF16)
    mks = pp.tile([128, 128], BF16)
    ident = pp.tile([128, 128], BF16)
    warm = pp.tile([128, 64], BF16)
    xts = [pp.tile([128, L], BF16, name=f"xt{i}") for i in range(8)]
    if FP8_QK:
        x8 = pp.tile([128, 8, L], FP8)
    qt_c = [pp.tile([128, 2, qw], BF16, name=f"qt{i}") for i in range(n_qch)]
    kt_c = [pp.tile([128, 2, qw], BF16, name=f"ktc{i}") for i in range(n_qch)]
    vt_c = [pp.tile([128, qw // 128, C + 4], BF16, name=f"vt{i}")
            for i in range(n_qch)]
    at = pp.tile([128, 2, L], BF16)

    # ---- loads: weights/tables first, x column-chunk-major for early start
    nc.vector.memset(warm[:], 0.0)
    nc.sync.dma_start(out=wq[:], in_=wq_d.rearrange("(a p) c -> p a c", p=128))
    nc.sync.dma_start(out=wk[:], in_=wk_d.rearrange("(a p) c -> p a c", p=128))
    if FP8_QK:
        x8r = x8_d.rearrange("(a p) l -> p a l", p=128)
        for lc in range(n_qch):
            nc.sync.dma_start(out=x8[:, :, lc * qw:lc * qw + qw],
                              in_=x8r[:, :, lc * qw:lc * qw + qw])
    for lc in range(n_qch):
        for dt_ in range(8):
            nc.sync.dma_start(out=xts[dt_][:, lc * qw:lc * qw + qw],
                              in_=xt_d[dt_ * 128:dt_ * 128 + 128,
                                       lc * qw:lc * qw + qw])
        if lc == 0:
            nc.sync.dma_start(out=cs[:], in_=cos_d)
            nc.sync.dma_start(out=sn[:], in_=sin_d)
            nc.sync.dma_start(
                out=wv[:], in_=wv_d.rearrange("(a p) c -> p a c", p=128))
            nc.sync.dma_start(out=mks[:], in_=mk_d)
            nc.sync.dma_start(out=ident[:], in_=id_d)
        if lc == 1:
            nc.sync.dma_start(
                out=wo[:], in_=wo_d.rearrange("(a p) e -> p a e", p=128))
    for i in range(n_qch):
        ov = vt_c[i][:, :, :].rearrange("p l (h x) -> p l h x", x=65)
        nc.gpsimd.memset(ov[:, :, :, 64], 1.0)

    with tc.tile_pool(name="proj_ps", bufs=2, space="PSUM") as projp, \
         tc.tile_pool(name="sc_ps", bufs=2, space="PSUM") as scp, \
         tc.tile_pool(name="po_ps", bufs=2, space="PSUM") as pop:
        # warm-up: span the PE p-state ramp while input DMAs land
        wps = projp.tile([128, 64], F32, tag="proj", name="warm_ps")
        for r in range(180):
            nc.tensor.matmul(wps[0:64, :], lhsT=warm[:], rhs=warm[:],
                             start=(r == 0), stop=(r == 179))

        def gen_proj(qc):
            """Generator emitting chunk qc's Q/K/V projection; yields after
            each tensor-engine matmul so callers can interleave."""
            ls = qc * qw
            for ct in (0, 1):
                for nm, w, dstc in (("q", wq, qt_c), ("k", wk, kt_c)):
                    p = projp.tile([128, qw], F32, tag="proj",
                                   name=f"ps_{nm}{ct}_{qc}")
                    if FP8_QK:
                        for dp in range(4):
                            nc.tensor.matmul(
                                p[:],
                                lhsT=w[:, 2 * dp:2 * dp + 2,
                                       ct * 128:ct * 128 + 128],
                                rhs=x8[:, 2 * dp:2 * dp + 2, ls:ls + qw],
                                perf_mode=mybir.MatmulPerfMode.DoubleRow,
                                start=(dp == 0), stop=(dp == 3))
                            yield
                    else:
                        for dt_ in range(8):
                            nc.tensor.matmul(
                                p[:],
                                lhsT=w[:, dt_, ct * 128:ct * 128 + 128],
                                rhs=xts[dt_][:, ls:ls + qw],
                                start=(dt_ == 0), stop=(dt_ == 7))
                            yield
                    sh = rtp.tile([128, qw], F32, tag="t",
                                  name=f"sh_{nm}{ct}{qc}")
                    t1 = rtp.tile([128, qw], F32, tag="t",
                                  name=f"t1_{nm}{ct}{qc}")
                    t2 = rtp.tile([128, qw], F32, tag="t",
                                  name=f"t2_{nm}{ct}{qc}")
                    nc.vector.stream_shuffle(sh[:], p[:], SWAP32)
                    nc.vector.tensor_mul(t1[:], p[:], cs[:, ls:ls + qw])
                    nc.gpsimd.tensor_mul(t2[:], sh[:], sn[:, ls:ls + qw])
                    nc.gpsimd.tensor_add(dstc[qc][:, ct, :], t1[:], t2[:])
            for lt in range(ls // 128, (ls + qw) // 128):
                pv = projp.tile([128, C], F32, tag="proj", name=f"pv_{lt}")
                for dt_ in range(8):
                    nc.tensor.matmul(
                        pv[:],
                        lhsT=xts[dt_][:, lt * 128:lt * 128 + 128],
                        rhs=wv[:, dt_, :],
                        start=(dt_ == 0), stop=(dt_ == 7))
                    yield
                ov = vt_c[lt // 4][:, lt % 4, :].rearrange(
                    "p (h x) -> p h x", x=65)[:, :, 0:64]
                nc.vector.tensor_copy(ov,
                                      pv[:].rearrange("p (h x) -> p h x",
                                                      x=64))

        def gen_outproj(qc):
            """Generator emitting chunk qc's output projection; yields after
            each matmul. Runs as filler during chunk qc+1's attention."""
            for qt in range(4):
                qtl = 4 * qc + qt
                stg = osp.tile([128, 1024], BF16, tag="stg",
                               name=f"stg_{qtl}")
                for eh in range(2):
                    pout = projp.tile([128, 512], F32, tag="proj",
                                      name=f"pout_{qtl}_{eh}")
                    for ct in range(2):
                        nc.tensor.matmul(
                            pout[:],
                            lhsT=at[:, ct, qtl * 128:qtl * 128 + 128],
                            rhs=wo[:, ct, eh * 512:eh * 512 + 512],
                            start=(ct == 0), stop=(ct == 1),
                            skip_group_check=True)
                        yield
                    if eh == 0:
                        nc.vector.tensor_copy(stg[:, 0:512], pout[:])
                    else:
                        nc.scalar.copy(stg[:, 512:1024], pout[:])
                nc.sync.dma_start(out=out_d[qtl * 128:qtl * 128 + 128, :],
                                  in_=stg[:])

        import itertools as _it
        for _ in gen_proj(0):
            pass
        for qc in range(n_qch):
            ls = qc * qw
            # filler: next chunk's projection + deferred out-projections
            # (out-proj of chunk c runs during attention of chunk c+2, so the
            # last, largest attention chunk gets two out-proj generators)
            nproj = (64 if not FP8_QK else 48) if qc + 1 < n_qch else 0
            ops = {2: [0], 3: [1, 2]}.get(qc, [])
            nop = 16 * len(ops)
            gens = []
            if nproj:
                gens.append(gen_proj(qc + 1))
            for oc in ops:
                gens.append(gen_outproj(oc))
            filler = _it.chain(*gens)
            n_iter = 2 * (4 * qc + 4 + 4)
            fleft = nproj + nop
            it_left = n_iter

            def drain():
                nonlocal fleft, it_left
                k = -(-fleft // it_left) if it_left > 0 else fleft
                for _ in range(k):
                    try:
                        next(filler)
                    except StopIteration:
                        fleft = 0
                        break
                    fleft -= 1
                it_left -= 1

            # ---- attention for this q chunk
            n_kt = 4 * qc + 4
            for pair in range(2):
                a2s = [a2p.tile([128, 128], BF16, tag="a2",
                                name=f"a2_{qc}_{pair}_{qt}")
                       for qt in range(4)]
                po_h = [pop.tile([128, 4, 65], F32, tag="po",
                                 name=f"po_{qc}_{pair}_{hl}")
                        for hl in range(2)]
                p8_hist = {}
                po_started = [False, False]

                def emit_pv(hloc, qt, pkt):
                    h = 2 * pair + hloc
                    nc.tensor.matmul(
                        po_h[hloc][:, qt, :],
                        lhsT=p8_hist[pkt][:, hloc,
                                          qt * 128:qt * 128 + 128],
                        rhs=vt_c[pkt // 4][:, pkt % 4, 65 * h:65 * h + 65],
                        start=not po_started[hloc],
                        stop=(pkt == 4 * qc + qt),
                        skip_group_check=True)
                    po_started[hloc] = True

                def emit_norm(hloc, qt):
                    rin = rip.tile([128, 1], F32, tag="ri",
                                   name=f"ri_{qc}_{pair}_{hloc}_{qt}")
                    nc.vector.reciprocal_approx_fast(
                        out=rin[:], in_=po_h[hloc][:, qt, 64:65])
                    nc.vector.tensor_scalar_mul(
                        a2s[qt][:, 64 * hloc:64 * hloc + 64],
                        po_h[hloc][:, qt, 0:64],
                        rin[:, 0:1])

                for kt in range(n_kt + 4):
                    if kt < n_kt:
                        off = kt * 128 - ls
                        qlo = max(0, off)
                        kc, ko = kt // 4, (kt % 4) * 128
                        sc = scp.tile([128, 2, qw], F32, tag="sc",
                                      name=f"sc_{qc}_{pair}_{kt}")
                        for hloc in range(2):
                            nc.tensor.matmul(
                                sc[:, hloc, qlo:qw],
                                lhsT=kt_c[kc][64 * hloc:64 * hloc + 64, pair,
                                              ko:ko + 128],
                                rhs=qt_c[qc][64 * hloc:64 * hloc + 64, pair,
                                             qlo:qw],
                                start=True, stop=True,
                                tile_position=(64 * hloc, 0),
                                skip_group_check=True)
                        p8 = ptp.tile([128, 2, qw], BF16, tag="p",
                                      name=f"p8_{qc}_{pair}_{kt}")
                        nc.scalar.activation(p8[:, :, qlo:qw],
                                             sc[:, :, qlo:qw],
                                             mybir.ActivationFunctionType.Exp,
                                             scale=scale)
                        if off >= 0:
                            for hloc in range(2):
                                nc.gpsimd.tensor_mul(
                                    p8[:, hloc, qlo:qlo + 128],
                                    p8[:, hloc, qlo:qlo + 128],
                                    mks[:, 0:128])
                        p8_hist[kt] = p8
                    # non-diagonal PV lags scores by 2 (exp already done);
                    # the masked diagonal-block PV lags by 4 for mask slack
                    pkt = kt - 2
                    if 0 <= pkt < n_kt:
                        poff = pkt * 128 - ls
                        for hloc in range(2):
                            for qt in range(4):
                                if qt * 128 > poff:
                                    emit_pv(hloc, qt, pkt)
                    dkt = kt - 4
                    if dkt >= 4 * qc:
                        dqt = dkt - 4 * qc  # diagonal q tile of kt dkt
                        for hloc in range(2):
                            emit_pv(hloc, dqt, dkt)
                            emit_norm(hloc, dqt)  # dqt's group just closed
                        del p8_hist[dkt]
                    elif kt >= 4:
                        del p8_hist[kt - 4]
                    drain()
                # transpose pair's A [q, 2x64d] -> at [128ch, q]
                for qt in range(4):
                    tp = pop.tile([128, 128], BF16, tag="po",
                                  name=f"tp_{qc}_{pair}_{qt}")
                    nc.tensor.matmul(tp[:], lhsT=a2s[qt][:], rhs=ident[:],
                                     is_transpose=True,
                                     skip_group_check=True)
                    nc.vector.tensor_copy(
                        at[:, pair, ls + qt * 128:ls + qt * 128 + 128], tp[:])

            for _ in filler:  # emit any tail (e.g. last V copy)
                pass
        # ---- last chunk's output projection
        for _ in gen_outproj(n_qch - 1):
            pass


def build_nc(L=2048):
    """Build + compile the per-core Bass program (same NEFF on all 8 cores)."""
    assert L % 512 == 0
    nc = bacc.Bacc("TRN2", target_bir_lowering=False, debug=False,
                   num_devices=N_CORES)
    with tile.TileContext(nc) as tc:
        with tc.tile_pool(name="persist", bufs=1) as pp, \
             tc.tile_pool(name="ropet", bufs=9) as rtp, \
             tc.tile_pool(name="ptp", bufs=7) as ptp, \
             tc.tile_pool(name="a2p", bufs=10) as a2p, \
             tc.tile_pool(name="rinvp", bufs=6) as rip, \
             tc.tile_pool(name="ostg", bufs=3) as osp:
            _body(nc, tc, L, pp, rtp, ptp, a2p, rip, osp)
    nc.compile()
    return nc


_NC_CACHE = {}


def _get_nc(L):
    if L not in _NC_CACHE:
        _NC_CACHE[L] = build_nc(L)
    return _NC_CACHE[L]


def make_inputs(x, token_positions, Wq, Wk, Wv, Wo):
    """Host-side shard/layout prep -> list of 8 per-core input dicts."""
    B, L, _ = x.shape
    pos = np.asarray(token_positions).astype(np.float64)
    S = ROPE_THETA ** (-2.0 / D_HEAD)
    thetas = S ** np.arange(HALF, dtype=np.float64)
    ang = pos[:, None] * thetas[None, :]          # [L, 32]
    cosL = np.cos(ang).T                          # [32, L]
    sinL = np.sin(ang).T
    # per-channel tables on the natural (head, dim) layout:
    # row p (within a 64-row head block): pair i = (p%64)//2
    # cosb[p] = cos(theta_i * pos); ssin[p] = -sin if dim even else +sin
    cosb = np.empty((128, L), dtype=np.float64)
    ssin = np.empty((128, L), dtype=np.float64)
    for p in range(128):
        i = (p % 64) // 2
        cosb[p] = cosL[i]
        ssin[p] = -sinL[i] if (p % 2 == 0) else sinL[i]
    cosb = cosb.astype(NPBF16)
    ssin = ssin.astype(NPBF16)

    r = np.arange(128)[:, None]
    col = np.arange(128)[None, :]
    masks = (col >= r).astype(NPBF16)  # [128, 128] tril(keep q>=k)
    ident = np.eye(128).astype(NPBF16)

    wdt = NPFP8 if FP8_QK else NPBF16
    xts = [np.ascontiguousarray(x[b].astype(NPBF16).T) for b in range(B)]
    if FP8_QK:
        xt8s = [np.ascontiguousarray(x[b].astype(NPFP8).T) for b in range(B)]
    in_maps = []
    shard_cache = {}
    for core in range(N_CORES):
        b, hg = core // 4, core % 4
        if hg not in shard_cache:
            rows = slice(hg * 256, hg * 256 + 256)
            shard_cache[hg] = {
                "wqt": np.ascontiguousarray(Wq[rows].astype(wdt).T),
                "wkt": np.ascontiguousarray(Wk[rows].astype(wdt).T),
                "wvt": np.ascontiguousarray(Wv[rows].astype(NPBF16).T),
                "wot": np.ascontiguousarray(Wo[:, rows].astype(NPBF16).T),
            }
        m = dict(shard_cache[hg])
        m["xt"] = xts[b]
        if FP8_QK:
            m["xt8"] = xt8s[b]
        m["cosb"] = cosb
        m["ssin"] = ssin
        m["masks"] = masks
        m["ident"] = ident
        in_maps.append(m)
    return in_maps


def kernel(x, token_positions, Wq, Wk, Wv, Wo):
    x = np.asarray(x); Wq = np.asarray(Wq); Wk = np.asarray(Wk)
    Wv = np.asarray(Wv); Wo = np.asarray(Wo)
    B, L, _ = x.shape
    nc = _get_nc(L)
    in_maps = make_inputs(x, token_positions, Wq, Wk, Wv, Wo)
    res = run_bass_kernel_spmd(nc, in_maps, core_ids=list(range(N_CORES)))
    out = np.zeros((B, L, D_MODEL), dtype=np.float32)
    for core in range(N_CORES):
        out[core // 4] += res.results[core]["out"].astype(np.float32)
    return out
